# revision 20
# baseline (speedup 1.0000x reference)
"""Single-head attention (B=4, S=4096, Dm=512, Dh=64) on 8 TRN2 NeuronCores.

Sharding: core c -> (batch b = c//2, query-half h = c%2). Each core receives
x[b]^T with its query half rolled to the front, projects q (its 2048 rows) and
k/v (all 4096 rows), and runs flash-style attention entirely on-chip:

  scores^T tiles [keys=128p, qrows=512f] = kT_tile.T @ qT   (K=64 contraction,
      two key tiles packed concurrently on PE row-groups 0-1 / 2-3)
  P^T = exp(scores^T / 8)                                   (ScalarE, fused scale)
  out^T[65, qrows] += vaug_tile.T @ P^T_tile                (vaug = [v | 1]: row 64
      accumulates the softmax denominator for free)
  out = out^T[0:64] * broadcast(1 / out^T[64])

The kernel returns out^T (64, 2048) per core; the host transposes/gathers.
Key order within a core is rolled, which softmax+PV is invariant to.

Scheduling notes: all staging tensors are split into 512-column chunk tiles so
Tile's dependency tracking never serializes on a whole tensor; the x^T load is
column-chunked on the SP HWDGE ring while small duplication/output DMAs ride
the gpsimd SWDGE ring; the second-half projections are woven into attention
chunk 0 so the ScalarE exp stream (the chip bottleneck) starts ~8us in.
"""

import numpy as np

from concourse import bacc, bass, mybir
from concourse.bass_utils import run_bass_kernel_spmd
from concourse.masks import make_identity
from concourse.tile import TileContext

B, S, DM, DH = 4, 4096, 512, 64
SQ = S // 2          # query rows per core
KT = S // 128        # 32 key tiles
NP = KT // 2         # 16 score pairs per query chunk
F32 = mybir.dt.float32
F32R = mybir.dt.float32r
BF16 = mybir.dt.bfloat16
EXP = mybir.ActivationFunctionType.Exp


def build_nc(pss_bufs=2, po_bufs=2, pab_bufs=2, pt_bufs=4):
    nc = bacc.Bacc("TRN2", debug=False)

    xT_ext = nc.declare_dram_parameter("xT", [DM, S], BF16, isOutput=False)
    wqk_ext = nc.declare_dram_parameter("wqk", [DM, 128], BF16, isOutput=False)
    wv_ext = nc.declare_dram_parameter("wv", [DM, DH], BF16, isOutput=False)
    bqk_ext = nc.declare_dram_parameter("bqk", [128, 1], F32, isOutput=False)
    bkq_ext = nc.declare_dram_parameter("bkq", [128, 1], F32, isOutput=False)
    bkk_ext = nc.declare_dram_parameter("bkk", [128, 1], F32, isOutput=False)
    bv_ext = nc.declare_dram_parameter("bv", [DH, 1], F32, isOutput=False)
    out_ext = nc.declare_dram_parameter("out", [DH, SQ], F32, isOutput=True)

    with TileContext(nc) as tc:
        with (
            tc.tile_pool(name="persist", bufs=1) as persist,
            tc.tile_pool(name="stage", bufs=3) as stage,
            tc.tile_pool(name="psAB", bufs=pab_bufs, space="PSUM") as psAB,
            tc.tile_pool(name="psS", bufs=pss_bufs, space="PSUM") as psS,
            tc.tile_pool(name="psO", bufs=po_bufs, space="PSUM") as psO,
            tc.tile_pool(name="pT", bufs=pt_bufs) as pTp,
            tc.tile_pool(name="small", bufs=2) as small,
        ):
            # ---- persistent SBUF tensors (split per 512-column chunk) ----
            xT_sb = [persist.tile([128, 4, 512], BF16, tag=f"xT{n}", name=f"xT{n}") for n in range(8)]
            wqk_sb = persist.tile([128, 4, 128], BF16)
            wv_sb = persist.tile([128, 4, DH], BF16)
            bqk_sb = persist.tile([128, 1], F32)
            bkq_sb = persist.tile([128, 1], F32)
            bkk_sb = persist.tile([128, 1], F32)
            bv_sb = persist.tile([DH, 1], F32)
            ident = persist.tile([DH, DH], F32)
            ones = persist.tile([1, DH], F32R)
            ones_f = persist.tile([1, DH], F32)
            ones4_f = persist.tile([128, 4], F32)
            # qT (both halves duplicated), per query chunk
            qT2 = [persist.tile([128, 512], F32R, tag=f"qT{n}", name=f"qT{n}") for n in range(4)]
            # kT (both halves duplicated), per 512-key chunk (= 2 score pairs)
            kTc = [persist.tile([128, 512], F32R, tag=f"kT{n}", name=f"kT{n}") for n in range(8)]
            # v natural layout + ones column, per 512-key chunk (4 key tiles)
            vaugc = [persist.tile([128, 4, DH + 1], BF16, tag=f"va{n}", name=f"va{n}") for n in range(8)]
            vTc = [persist.tile([DH, 512], F32, tag=f"vT{n}", name=f"vT{n}") for n in range(8)]

            nc.sync.dma_start(out=wqk_sb[:], in_=wqk_ext.ap().rearrange("(k p) m -> p k m", p=128))
            nc.sync.dma_start(out=wv_sb[:], in_=wv_ext.ap().rearrange("(k p) m -> p k m", p=128))
            nc.sync.dma_start(out=bqk_sb[:], in_=bqk_ext.ap())
            nc.sync.dma_start(out=bkq_sb[:], in_=bkq_ext.ap())
            nc.sync.dma_start(out=bkk_sb[:], in_=bkk_ext.ap())
            nc.sync.dma_start(out=bv_sb[:], in_=bv_ext.ap())
            make_identity(nc, ident[:])
            nc.vector.memset(ones_f[:], 1.0)
            nc.vector.memset(ones4_f[:], 1.0)
            nc.vector.tensor_copy(ones[:], ones_f[:])
            for n in range(8):
                nc.vector.tensor_copy(vaugc[n][:, :, DH], ones4_f[:])

            # column-chunked x^T load; chunks are emitted just-in-time in the
            # weave below so small duplication DMAs are never queued behind
            # the whole bulk load on the shared SDMA engines
            xT_r = xT_ext.ap().rearrange("(k p) s -> p k s", p=128)
            xT_loaded = [False] * 8

            def xload(n):
                if not xT_loaded[n]:
                    xT_loaded[n] = True
                    nc.sync.dma_start(out=xT_sb[n][:], in_=xT_r[:, :, n * 512:(n + 1) * 512])

            # ---- projection units (small DMAs on the gpsimd SWDGE ring) ----
            def qkproj(n):
                # P = [q; k], Q = [k; q] — each built from col-tiled M=64
                # matmul pairs on disjoint PSUM partition halves, so every
                # evacuation is partition-aligned (no duplication DMAs).
                P = psAB.tile([128, 512], F32, tag="ab", name=f"qkP{n}")
                Q = psAB.tile([128, 512], F32, tag="ab", name=f"qkQ{n}")
                for k in range(4):
                    nc.tensor.matmul(
                        P[0:DH, :], wqk_sb[:, k, 0:DH], xT_sb[n][:, k, :],
                        start=(k == 0), stop=(k == 3), tile_position=(0, 0),
                        skip_group_check=True,
                    )
                    nc.tensor.matmul(
                        P[DH:128, :], wqk_sb[:, k, DH:128], xT_sb[n][:, k, :],
                        start=(k == 0), stop=(k == 3), tile_position=(0, 64),
                        skip_group_check=True,
                    )
                for k in range(4):
                    nc.tensor.matmul(
                        Q[0:DH, :], wqk_sb[:, k, DH:128], xT_sb[n][:, k, :],
                        start=(k == 0), stop=(k == 3), tile_position=(0, 0),
                        skip_group_check=True,
                    )
                    nc.tensor.matmul(
                        Q[DH:128, :], wqk_sb[:, k, 0:DH], xT_sb[n][:, k, :],
                        start=(k == 0), stop=(k == 3), tile_position=(0, 64),
                        skip_group_check=True,
                    )
                nc.vector.tensor_scalar_add(qT2[n][0:DH, :], P[0:DH, :], bqk_sb[0:DH])
                nc.vector.tensor_scalar_add(kTc[n][DH:128, :], P[DH:128, :], bqk_sb[DH:128])
                nc.vector.tensor_scalar_add(kTc[n][0:DH, :], Q[0:DH, :], bkq_sb[0:DH])
                nc.vector.tensor_scalar_add(qT2[n][DH:128, :], Q[DH:128, :], bkq_sb[DH:128])

            def kproj(n):
                """k projection of other-half chunk n -> kTc[4+n] = [k; k]."""
                ps = psAB.tile([128, 512], F32, tag="ab", name=f"k{n}")
                for k in range(4):
                    nc.tensor.matmul(
                        ps[0:DH, :], wqk_sb[:, k, DH:128], xT_sb[4 + n][:, k, :],
                        start=(k == 0), stop=(k == 3), tile_position=(0, 0),
                        skip_group_check=True,
                    )
                    nc.tensor.matmul(
                        ps[DH:128, :], wqk_sb[:, k, DH:128], xT_sb[4 + n][:, k, :],
                        start=(k == 0), stop=(k == 3), tile_position=(0, 64),
                        skip_group_check=True,
                    )
                nc.vector.tensor_scalar_add(kTc[4 + n][:], ps[:], bkk_sb[:])

            def vproj(n):
                ps = psAB.tile([128, 512], F32, tag="ab", name=f"v{n}")
                for k in range(4):
                    nc.tensor.matmul(
                        ps[0:DH, :], wv_sb[:, k, :], xT_sb[n][:, k, :],
                        start=(k == 0), stop=(k == 3),
                    )
                nc.vector.tensor_scalar_add(vTc[n][:], ps[0:DH, :], bv_sb[:])

            def vtrans(n):
                for j in range(4):
                    pt = psAB.tile([128, 512], F32, tag="ab", name=f"tr{n}_{j}")
                    nc.tensor.transpose(pt[:, 0:DH], vTc[n][:, j * 128:(j + 1) * 128], ident[:])
                    nc.vector.tensor_copy(vaugc[n][:, j, 0:DH], pt[:, 0:DH])

            # ---- attention machinery ----
            def scores(qc, pr, pss):
                ps = psS.tile([128, 1024], F32, tag="s")
                c, o = divmod(pr, 2)
                nc.tensor.matmul(
                    ps[:, 0:512], kTc[c][0:DH, o * 256:o * 256 + 128], qT2[qc][0:DH, :],
                    start=True, stop=True, tile_position=(0, 0),
                )
                nc.tensor.matmul(
                    ps[:, 512:1024], kTc[c][DH:128, o * 256 + 128:o * 256 + 256], qT2[qc][DH:128, :],
                    start=True, stop=True, tile_position=(64, 0),
                )
                pss.append(ps)

            def pexp(pr, pss, pts):
                pt = pTp.tile([128, 1024], BF16, tag="pt")
                nc.scalar.activation(pt[:], pss[pr][:], EXP, scale=0.125)
                pts.append(pt)

            def pv(pr, po, pts):
                c, o = divmod(pr, 2)
                va = vaugc[c]
                nc.tensor.matmul(
                    po[0:DH + 1, :], va[:, 2 * o, :], pts[pr][:, 0:512],
                    start=(pr == 0), stop=False, skip_group_check=True,
                )
                nc.tensor.matmul(
                    po[0:DH + 1, :], va[:, 2 * o + 1, :], pts[pr][:, 512:1024],
                    start=False, stop=(pr == NP - 1), skip_group_check=True,
                )

            def epilogue(qc, po):
                den = small.tile([1, 512], F32, tag="den")
                nc.vector.tensor_copy(den[:], po[DH:DH + 1, :])
                rec = small.tile([1, 512], F32R, tag="rec")
                with nc.allow_low_precision(reason="softmax denom reciprocal"):
                    nc.vector.reciprocal(rec[:], den[:])
                pb = psAB.tile([128, 512], F32, tag="ab", name=f"bc{qc}")
                nc.tensor.matmul(pb[0:DH, :], ones[:], rec[:], start=True, stop=True)
                recb = small.tile([DH, 512], F32, tag="recb")
                nc.vector.tensor_copy(recb[:], pb[0:DH, :])
                osb = stage.tile([DH, 512], F32, tag="osb")
                nc.vector.tensor_mul(osb[:], po[0:DH, :], recb[:])
                nc.sync.dma_start(out=out_ext.ap()[:, qc * 512:qc * 512 + 512], in_=osb[:])

            # ---- emission: phase B woven into attention chunk 0 ----
            def unit_v(n):
                vproj(n)
                vtrans(n)

            xload(0)
            xload(1)
            qkproj(0)
            xload(2)
            unit_v(0)
            qkproj(1)
            xload(3)

            units = [
                lambda: unit_v(1),
                lambda: (qkproj(2), xload(4)),
                lambda: unit_v(2),
                lambda: (qkproj(3), xload(5)),
                lambda: unit_v(3),
                lambda: (kproj(0), xload(6)),
                lambda: unit_v(4),
                lambda: (kproj(1), xload(7)),
                lambda: unit_v(5),
                lambda: kproj(2),
                lambda: unit_v(6),
                lambda: kproj(3),
                lambda: unit_v(7),
            ]

            for qc in range(4):
                po = psO.tile([128, 512], F32, tag="po", name=f"po{qc}")
                pss, pts = [], []
                scores(qc, 0, pss)
                scores(qc, 1, pss)
                for pr in range(NP):
                    pexp(pr, pss, pts)
                    if qc == 0 and pr < len(units):
                        units[pr]()
                    if pr + 2 < NP:
                        scores(qc, pr + 2, pss)
                    pv(pr, po, pts)
                epilogue(qc, po)

    nc.finalize()
    return nc


_NC = None


def _get_nc():
    global _NC
    if _NC is None:
        _NC = build_nc()
    return _NC


def _prep_in_maps(x, Wq, bq, Wk, bk, Wv, bv):
    import ml_dtypes
    bf16 = ml_dtypes.bfloat16
    wqk = np.ascontiguousarray(np.concatenate([Wq, Wk], axis=1)).astype(bf16)
    wv = np.ascontiguousarray(Wv).astype(bf16)
    bqk = np.concatenate([bq, bk]).reshape(128, 1).astype(np.float32)
    bkq = np.concatenate([bk, bq]).reshape(128, 1).astype(np.float32)
    bkk = np.concatenate([bk, bk]).reshape(128, 1).astype(np.float32)
    bv_c = np.ascontiguousarray(np.asarray(bv).reshape(DH, 1), dtype=np.float32)
    in_maps = []
    for c in range(8):
        b, h = divmod(c, 2)
        xT = x[b].T  # (DM, S)
        if h == 1:
            xT = np.concatenate([xT[:, SQ:], xT[:, :SQ]], axis=1)
        in_maps.append({
            "xT": np.ascontiguousarray(xT).astype(bf16),
            "wqk": wqk, "wv": wv, "bqk": bqk, "bkq": bkq, "bkk": bkk, "bv": bv_c,
        })
    return in_maps


def kernel(x, mask, Wq, bq, Wk, bk, Wv, bv):
    x = np.asarray(x)
    mask = np.asarray(mask)
    if not np.all(mask != 0):
        # Faithful fallback (mask applied after softmax with -inf fill) — the
        # spec fills mask with ones, so this path should never run.
        q = x @ np.asarray(Wq) + np.asarray(bq)
        k = x @ np.asarray(Wk) + np.asarray(bk)
        v = x @ np.asarray(Wv) + np.asarray(bv)
        s = np.einsum("bqd,bkd->bqk", q, k) / np.sqrt(np.float32(DH))
        s = s - s.max(axis=-1, keepdims=True)
        e = np.exp(s)
        a = e / e.sum(axis=-1, keepdims=True)
        a = np.where(mask[:, :, :] == 0, -np.inf, a)
        return np.einsum("bqk,bkd->bqd", a, v).astype(np.float32)

    nc = _get_nc()
    in_maps = _prep_in_maps(x, Wq, bq, Wk, bk, Wv, bv)
    res = run_bass_kernel_spmd(nc, in_maps, list(range(8)))
    y = np.empty((B, S, DH), dtype=np.float32)
    for c in range(8):
        b, h = divmod(c, 2)
        y[b, h * SQ:(h + 1) * SQ, :] = res.results[c]["out"].T
    return y
